# revision 11
# baseline (speedup 1.0000x reference)
"""Trainium2 Bass kernel for DiffusionPropers (gnn_message_passing).

Strategy: shard the 100K propers across 8 NeuronCores (12544 each incl.
pads).  Per core:
  - The raw `encoded` table (25088 atoms x 128 bf16) lives in SBUF.
  - Per 896-proper chunk: 4 SBUF-source TRANSPOSED dma_gathers fetch the
    endpoint features straight into feature-major layout [128d, 896p]
    (no on-chip transpose needed); the gathers round-robin over the 4
    SWDGE queues so their Q7 descriptor-gen runs concurrently.
  - W0 is folded on-chip: ZT = sum_k W0_k^T @ Ek^T (PE), the geometry
    part enters via a 16-row wmisc matmul, the 128->128->128->2 MLP runs
    feature-major at 448-col granularity in bf16 with Prelu (alpha=1e-3)
    fused into the PSUM evacuations.
  - Per-proper coords are streamed densely from the host (layout prep
    only); dihedral geometry (sin/cos via rsqrt identity, no arctan) is
    computed on DVE in proper-major layout.
  - Corrections dma_scatter_add into 4 rotating DRAM accumulators
    (avoids the WAW serialization of consecutive scatters into one
    tensor).  Race-freedom: the host orders propers so each 896-op
    chunk has all-distinct targets per endpoint; concurrent scatters
    always hit different accumulator tensors.
Host: sums the accumulators of all cores into `answer` (the all-reduce).
"""
import numpy as np
import ml_dtypes

# ---------------- compile-time constants (hardcoded problem shape) --------
N_ATOMS = 25000
NA = 25088              # padded atoms (196 * 128)
P_TOT = 100000
T_STEPS = 4
D = 128
N_CORES = 8
PPC = 12500             # real props per core
PPCT = 12544            # padded props per core (98 tiles of 128)
CH = 896                # props per gather/scatter call (SWDGE ring limit)
NCHUNK = PPCT // CH     # 14
CBLK = CH // 128        # 7
DUMP = NA               # scatter dump row
A_ROWS = NA + 8         # accumulator rows (incl. dump)
A_COLS = 64             # 256B stride for scatter
NACC = 4                # rotating scatter accumulators
LEAKY = 0.001

_BF16 = ml_dtypes.bfloat16

_compiled = None        # cached nc


# ------------------------- host-side helpers ------------------------------

def _wrap_idxs(idx: np.ndarray) -> np.ndarray:
    """[n] int -> [128, n/16] int16, wrapped in 16 partitions, replicated x8."""
    n = idx.shape[0]
    assert n % 16 == 0
    w = idx.reshape(-1, 16).T.astype(np.int16)
    return np.tile(w, (8, 1))


def _order_props(props: np.ndarray, n_real: int, seed: int = 0) -> np.ndarray:
    """Order PPCT props (rows of `props`, first n_real real) so that within
    every aligned CH-chunk the p0 targets are distinct and the p3 targets are
    distinct.  Pads (rows >= n_real) are unconstrained fillers (their scatter
    indices point at the dump row).  Returns a permutation of length PPCT."""
    n = props.shape[0]
    rng = np.random.default_rng(seed)
    for attempt in range(50):
        perm = rng.permutation(n_real)
        buckets: list[list[int]] = [[] for _ in range(NCHUNK)]
        used0: list[set] = [set() for _ in range(NCHUNK)]
        used3: list[set] = [set() for _ in range(NCHUNK)]
        fail = []
        start = 0
        for j in perm:
            a0 = int(props[j, 0])
            a3 = int(props[j, 3])
            for d in range(NCHUNK):
                b = (start + d) % NCHUNK
                if (len(buckets[b]) < CH and a0 not in used0[b]
                        and a3 not in used3[b]):
                    buckets[b].append(int(j))
                    used0[b].add(a0)
                    used3[b].add(a3)
                    break
            else:
                fail.append(int(j))
            start = (start + 1) % NCHUNK
        if fail:
            continue
        pads = list(range(n_real, n))
        for b in range(NCHUNK):
            while len(buckets[b]) < CH:
                buckets[b].append(pads.pop())
        assert not pads
        order = [j for b in buckets for j in b]
        return np.array(order, dtype=np.int64)
    raise RuntimeError("prop ordering failed")


# ------------------------- device kernel build ----------------------------

def _build():
    import concourse.bass as bass
    import concourse.bacc as bacc
    import concourse.mybir as mybir
    import concourse.tile as tile
    from concourse.masks import make_identity
    from concourse.library_config import mlp as mlp_lib

    F32 = mybir.dt.float32
    BF16 = mybir.dt.bfloat16
    I16 = mybir.dt.int16
    AF = mybir.ActivationFunctionType

    nc = bacc.Bacc("TRN2", target_bir_lowering=False, debug=False,
                   num_devices=N_CORES, num_swdge_queues=4)

    # ---- I/O ----
    encw = nc.dram_tensor("encw", [128, 196 * 128], BF16, kind="ExternalInput")
    cprop = nc.dram_tensor("cprop", [128, 98, 48], F32, kind="ExternalInput")
    w0all = nc.dram_tensor("w0all", [D, 512], BF16, kind="ExternalInput")
    wmisc = nc.dram_tensor("wmisc", [16, 512], BF16, kind="ExternalInput")
    w1 = nc.dram_tensor("w1", [D, D], BF16, kind="ExternalInput")
    w2 = nc.dram_tensor("w2", [D, D], BF16, kind="ExternalInput")
    w3s = nc.dram_tensor("w3s", [D, 2], BF16, kind="ExternalInput")
    bias12 = nc.dram_tensor("bias12", [D, 2], F32, kind="ExternalInput")
    b3h = nc.dram_tensor("b3h", [D, 2], F32, kind="ExternalInput")
    gidx = nc.dram_tensor("gidx", [128, 4 * (PPCT // 16)], I16, kind="ExternalInput")
    sidx = nc.dram_tensor("sidx", [128, 2 * (PPCT // 16)], I16, kind="ExternalInput")
    AA = [nc.dram_tensor(f"AA{q}", [A_ROWS, A_COLS], F32, kind="ExternalOutput")
          for q in range(NACC)]

    GI = PPCT // 16     # 784: idx columns per endpoint
    CGI = CH // 16      # 56: idx columns per chunk

    with tile.TileContext(nc) as tc:
        with (
            tc.tile_pool(name="const", bufs=1) as cpool,
        ):
            nc.gpsimd.load_library(mlp_lib)

            # ---- constants ----
            ibf = cpool.tile([128, 128], BF16)
            make_identity(nc, ibf[:])
            if32 = cpool.tile([128, 128], F32)
            make_identity(nc, if32[:])
            id8 = cpool.tile([8, 8], F32)
            make_identity(nc, id8[:])
            zero_b = cpool.tile([128, 1], F32)
            nc.vector.memset(zero_b[:], 0.0)
            eps_b = cpool.tile([128, 1], F32)
            nc.vector.memset(eps_b[:], 1e-12)

            w0t = cpool.tile([D, 512], BF16)
            nc.sync.dma_start(out=w0t[:], in_=w0all[:])
            wmt = cpool.tile([16, 512], BF16)
            nc.sync.dma_start(out=wmt[:], in_=wmisc[:])
            w1t = cpool.tile([D, D], BF16)
            nc.sync.dma_start(out=w1t[:], in_=w1[:])
            w2t = cpool.tile([D, D], BF16)
            nc.sync.dma_start(out=w2t[:], in_=w2[:])
            w3t = cpool.tile([D, 2], BF16)
            nc.sync.dma_start(out=w3t[:], in_=w3s[:])
            b12t = cpool.tile([D, 2], F32)
            nc.sync.dma_start(out=b12t[:], in_=bias12[:])
            b3t = cpool.tile([D, 2], F32)
            nc.sync.dma_start(out=b3t[:], in_=b3h[:])
            gixt = cpool.tile([128, 4 * GI], I16)
            nc.sync.dma_start(out=gixt[:], in_=gidx[:])
            sixt = cpool.tile([128, 2 * GI], I16)
            nc.sync.dma_start(out=sixt[:], in_=sidx[:])

            # encoded table resident in SBUF: atom a -> partition a%128,
            # rank a//128, 256B per rank
            TB = cpool.tile([128, 196 * 128], BF16)
            nc.scalar.dma_start(out=TB[:], in_=encw[:])
            # all per-proper coords [c0|c1|c2|c3] f32, exec order
            cpall = cpool.tile([128, 98, 48], F32)
            nc.sync.dma_start(out=cpall[:], in_=cprop[:])

            # ================= main loop: 14 chunks, software-pipelined ====
            with (
                tc.tile_pool(name="mn", bufs=2) as mpool,
                tc.tile_pool(name="geo", bufs=2) as gpool,
                tc.tile_pool(name="cto", bufs=4) as ctpool,
                tc.tile_pool(name="psb", bufs=1, space="PSUM") as psb,   # big: [128,1024]
                tc.tile_pool(name="pss", bufs=1, space="PSUM") as pss,   # small
            ):
                Gof = {}
                ctof = {}

                def do_gather(c):
                    G = []
                    for k in range(4):
                        g = mpool.tile([128, 1, CH], BF16, tag=f"g{k}", bufs=4)
                        nc.gpsimd.dma_gather(
                            g[:], TB[:],
                            gixt[:, k * GI + c * CGI:k * GI + (c + 1) * CGI],
                            CH, CH, 128, transpose=True,
                            sbuf_tokens_per_rank=128,
                            sbuf_free_dim_per_rank=256,
                            queue_num=(c + k) % 4)
                        G.append(g)
                    Gof[c] = G

                def do_compute(c):
                    G = Gof.pop(c)
                    cco = [cpall[:, c * CBLK:(c + 1) * CBLK, 12 * k:12 * (k + 1)]
                           for k in range(4)]
                    u1 = gpool.tile([128, CBLK, 12], F32, tag="u1")
                    u2 = gpool.tile([128, CBLK, 12], F32, tag="u2")
                    u3 = gpool.tile([128, CBLK, 12], F32, tag="u3")
                    dr = gpool.tile([128, CBLK, 12], F32, tag="dr")
                    nc.vector.tensor_sub(u1[:], cco[1], cco[0])
                    nc.vector.tensor_sub(u2[:], cco[2], cco[1])
                    nc.vector.tensor_sub(u3[:], cco[3], cco[2])
                    nc.vector.tensor_sub(dr[:], cco[0], cco[3])

                    def cross(out, a, b):
                        tmp = gpool.tile([128, CBLK, 4], F32, tag="ctmp")
                        for x in range(3):
                            y, z = (x + 1) % 3, (x + 2) % 3
                            sx, sy, sz = (slice(4 * v, 4 * v + 4) for v in (x, y, z))
                            nc.vector.tensor_mul(tmp[:], a[:, :, sy], b[:, :, sz])
                            nc.vector.tensor_mul(out[:, :, sx], a[:, :, sz], b[:, :, sy])
                            nc.vector.tensor_sub(out[:, :, sx], tmp[:], out[:, :, sx])

                    cr12 = gpool.tile([128, CBLK, 12], F32, tag="cr12")
                    cr23 = gpool.tile([128, CBLK, 12], F32, tag="cr23")
                    cross(cr12, u1, u2)
                    cross(cr23, u2, u3)

                    def dot3(out, a, b, tmp):
                        nc.vector.tensor_mul(tmp[:], a[:], b[:])
                        nc.vector.tensor_add(out[:], tmp[:, :, 0:4], tmp[:, :, 4:8])
                        nc.vector.tensor_add(out[:], out[:], tmp[:, :, 8:12])

                    tmp12 = gpool.tile([128, CBLK, 12], F32, tag="tmp12")
                    n2 = gpool.tile([128, CBLK, 4], F32, tag="n2")
                    dot3(n2, u2, u2, tmp12)
                    nc.scalar.activation(n2[:], n2[:], AF.Sqrt, bias=zero_b[:])
                    sn = gpool.tile([128, CBLK, 4], F32, tag="sn")
                    dot3(sn, u1, cr23, tmp12)
                    nc.vector.tensor_mul(sn[:], sn[:], n2[:])
                    cn = gpool.tile([128, CBLK, 4], F32, tag="cn")
                    dot3(cn, cr12, cr23, tmp12)
                    hy = gpool.tile([128, CBLK, 4], F32, tag="hy")
                    t2 = gpool.tile([128, CBLK, 4], F32, tag="t2")
                    nc.vector.tensor_mul(hy[:], sn[:], sn[:])
                    nc.vector.tensor_mul(t2[:], cn[:], cn[:])
                    nc.vector.tensor_add(hy[:], hy[:], t2[:])
                    nc.scalar.activation(hy[:], hy[:], AF.Sqrt, bias=eps_b[:])
                    rh = gpool.tile([128, CBLK, 4], F32, tag="rh")
                    nc.vector.reciprocal(rh[:], hy[:])
                    dl = gpool.tile([128, CBLK, 4], F32, tag="dl")
                    dot3(dl, dr, dr, tmp12)
                    nc.scalar.activation(dl[:], dl[:], AF.Sqrt, bias=eps_b[:])
                    rdl = gpool.tile([128, CBLK, 4], F32, tag="rdl")
                    nc.vector.reciprocal(rdl[:], dl[:])
                    dh = gpool.tile([128, CBLK, 12], F32, tag="dh")
                    for x in range(3):
                        nc.vector.tensor_mul(dh[:, :, 4 * x:4 * x + 4],
                                             dr[:, :, 4 * x:4 * x + 4], rdl[:])
                    # geo layout: rows f*4+ti after transpose (f: sin,cos,dl,one)
                    geo = gpool.tile([128, CBLK, 16], F32, tag="geo")
                    nc.vector.memset(geo[:], 1.0)
                    nc.vector.tensor_mul(geo[:, :, 0:4], sn[:], rh[:])
                    nc.vector.tensor_mul(geo[:, :, 4:8], cn[:], rh[:])
                    nc.vector.tensor_copy(geo[:, :, 8:12], dl[:])

                    # geoT [16, 896] via per-b PE transposes
                    gtb = gpool.tile([16, CH], BF16, tag="gtb")
                    for b in range(CBLK):
                        gtp = pss.tile([16, 128], F32, tag="gt")
                        nc.tensor.matmul(gtp[:], lhsT=geo[:, b, :], rhs=if32[:],
                                         is_transpose=True, start=True, stop=True)
                        nc.vector.tensor_copy(gtb[:, b * 128:(b + 1) * 128],
                                              gtp[:])

                    # ZT = sum_k W0_k^T @ Ek^T  -> [128, 896] bf16 (960 layout)
                    BLKS = [slice(0, 512), slice(512, 896)]
                    zps = psb.tile([128, 1024], F32, tag="hA")
                    for sl in BLKS:
                        for k in range(4):
                            nc.tensor.matmul(zps[:, sl],
                                             lhsT=w0t[:, k * 128:(k + 1) * 128],
                                             rhs=G[k][:, 0, sl],
                                             start=(k == 0), stop=(k == 3))
                    ztb = mpool.tile([128, CH], BF16, tag="ztb")
                    nc.vector.tensor_copy(ztb[:], zps[:, 0:CH])

                    # MLP per ti at 448-col granularity (960-wide tiles with
                    # a dead 448:512 gap so one ACT op covers both banks)
                    x1 = mpool.tile([128, 4, CH], BF16, tag="x1")
                    x2 = mpool.tile([128, 4, CH], BF16, tag="x2")
                    x3 = mpool.tile([128, 4, CH], BF16, tag="x3")
                    # dsb rows: s0_ti at partition 32*ti, s3_ti at 32*ti+1
                    # (quadrant-aligned partition offsets)
                    dsb = gpool.tile([128, CH], F32, tag="dsb")
                    for ti in range(4):
                        h1 = psb.tile([128, 1024], F32, tag="hB")
                        for sl in BLKS:
                            nc.tensor.matmul(h1[:, sl], lhsT=ibf[:],
                                             rhs=ztb[:, sl],
                                             start=True, stop=False)
                            nc.tensor.matmul(h1[:, sl],
                                             lhsT=wmt[:, ti * 128:(ti + 1) * 128],
                                             rhs=gtb[:, sl],
                                             start=False, stop=True)
                        nc.scalar.activation(x1[:, ti, :], h1[:, 0:CH], AF.Prelu,
                                             bias=zero_b[:], alpha=LEAKY)
                        h2 = psb.tile([128, 1024], F32, tag="hA")
                        for sl in BLKS:
                            nc.tensor.matmul(h2[:, sl], lhsT=w1t[:],
                                             rhs=x1[:, ti, sl],
                                             start=True, stop=True)
                        nc.scalar.activation(x2[:, ti, :], h2[:, 0:CH], AF.Prelu,
                                             bias=b12t[:, 0:1], alpha=LEAKY)
                        h3 = psb.tile([128, 1024], F32, tag="hB")
                        for sl in BLKS:
                            nc.tensor.matmul(h3[:, sl], lhsT=w2t[:],
                                             rhs=x2[:, ti, sl],
                                             start=True, stop=True)
                        nc.scalar.activation(x3[:, ti, :], h3[:, 0:CH], AF.Prelu,
                                             bias=b12t[:, 1:2], alpha=LEAKY)
                        for bi, sl in enumerate(BLKS):
                            dd = pss.tile([2, 512], F32, tag="dd", bufs=2)
                            nc.tensor.matmul(dd[:, 0:sl.stop - sl.start],
                                             lhsT=w3t[:], rhs=x3[:, ti, sl],
                                             start=True, stop=True)
                            eng = nc.vector if bi == 0 else nc.scalar
                            if bi == 0:
                                nc.vector.tensor_copy(
                                    dsb[32 * ti:32 * ti + 2, sl],
                                    dd[:, 0:512])
                            else:
                                nc.scalar.activation(
                                    dsb[32 * ti:32 * ti + 2, sl],
                                    dd[:, 0:384], AF.Copy)

                    # back to proper-major: dtc [128, 7, 8] (cols e*4+ti)
                    dtc = gpool.tile([128, CBLK, 8], F32, tag="dtc")
                    for b in range(CBLK):
                        dtpf = pss.tile([128, 128], F32, tag="dtp")
                        nc.tensor.matmul(dtpf[:],
                                         lhsT=dsb[:, b * 128:(b + 1) * 128],
                                         rhs=if32[:], is_transpose=True,
                                         start=True, stop=True)
                        nc.vector.tensor_copy(dtc[:, b, 0:4], dtpf[:, 0::32])
                        nc.vector.tensor_copy(dtc[:, b, 4:8], dtpf[:, 1::32])

                    c0t = ctpool.tile([128, CBLK, 12], F32, tag="c0t")
                    c3t = ctpool.tile([128, CBLK, 12], F32, tag="c3t")
                    s0 = gpool.tile([128, CBLK, 4], F32, tag="s0")
                    s3 = gpool.tile([128, CBLK, 4], F32, tag="s3")
                    nc.vector.tensor_scalar_add(s0[:], dtc[:, :, 0:4],
                                                b3t[:, 0:1])
                    nc.vector.tensor_scalar_add(s3[:], dtc[:, :, 4:8],
                                                b3t[:, 1:2])
                    for x in range(3):
                        xs = slice(4 * x, 4 * x + 4)
                        nc.vector.tensor_mul(c0t[:, :, xs], dh[:, :, xs], s0[:])
                        nc.vector.tensor_mul(c3t[:, :, xs], dh[:, :, xs], s3[:])
                    ctof[c] = (c0t, c3t)

                def do_scatter(c):
                    c0t, c3t = ctof.pop(c)
                    nc.gpsimd.dma_scatter_add(
                        AA[(2 * c) % NACC][:, :12], c0t[:],
                        sixt[:, c * CGI:(c + 1) * CGI],
                        CH, CH, 12, elem_step=A_COLS,
                        queue_num=1 + c % 3)
                    nc.gpsimd.dma_scatter_add(
                        AA[(2 * c + 1) % NACC][:, :12], c3t[:],
                        sixt[:, GI + c * CGI:GI + (c + 1) * CGI],
                        CH, CH, 12, elem_step=A_COLS,
                        queue_num=1 + (c + 1) % 3)

                for c in range(NCHUNK):
                    do_gather(c)
                    if c >= 1:
                        do_compute(c - 1)
                    if c >= 2:
                        do_scatter(c - 2)
                do_compute(NCHUNK - 1)
                do_scatter(NCHUNK - 2)
                do_scatter(NCHUNK - 1)

    nc.compile()
    return nc


def _get_compiled():
    global _compiled
    if _compiled is None:
        _compiled = _build()
    return _compiled


# ------------------------------ entry point -------------------------------

def _prep_in_maps(coords, propers, encoded, t, answer, W0, b0, W1, b1, W2, b2,
                  W3, b3):
    coords = np.asarray(coords, dtype=np.float32)
    propers_np = np.asarray(propers)
    encoded = np.asarray(encoded, dtype=np.float32)
    t = np.asarray(t, dtype=np.float32)
    W0 = np.asarray(W0, dtype=np.float32)
    b0 = np.asarray(b0, dtype=np.float32)
    W1 = np.asarray(W1, dtype=np.float32)
    b1 = np.asarray(b1, dtype=np.float32)
    W2 = np.asarray(W2, dtype=np.float32)
    b2 = np.asarray(b2, dtype=np.float32)
    W3 = np.asarray(W3, dtype=np.float32)
    b3 = np.asarray(b3, dtype=np.float32)

    # ---- shared (replicated) tensors ----
    encp = np.zeros((NA, D), dtype=_BF16)
    encp[:N_ATOMS] = encoded.astype(_BF16)
    # atom a -> partition a%128, rank a//128
    encw = np.ascontiguousarray(
        encp.reshape(196, 128, 128).transpose(1, 0, 2)).reshape(128, 196 * 128)

    # comp-major per-atom coords: cols = comp*4 + ti
    cflat = np.zeros((NA, 12), dtype=np.float32)
    cflat[:N_ATOMS] = coords.transpose(0, 2, 1).reshape(N_ATOMS, 12)

    w0all = np.concatenate([W0[128 * k:128 * (k + 1)] for k in range(4)],
                           axis=1).astype(_BF16)           # [128, 512]
    # geoT rows are f*4+ti (f: sin, cos, dl, one)
    wmisc = np.zeros((16, 512), dtype=np.float32)
    for ti in range(T_STEPS):
        wmisc[0 + ti, ti * 128:(ti + 1) * 128] = W0[513]
        wmisc[4 + ti, ti * 128:(ti + 1) * 128] = W0[514]
        wmisc[8 + ti, ti * 128:(ti + 1) * 128] = W0[515]
        wmisc[12 + ti, ti * 128:(ti + 1) * 128] = b0 + t[ti] * W0[512]
    wmisc = wmisc.astype(_BF16)
    w3s = np.stack([-0.5 * W3[:, 0], 0.5 * W3[:, 1]], axis=1).astype(_BF16)
    bias12 = np.stack([b1, b2], axis=1).astype(np.float32)  # [128, 2]
    b3h = np.zeros((D, 2), dtype=np.float32)
    b3h[:, 0] = -0.5 * b3[0]
    b3h[:, 1] = 0.5 * b3[1]

    shared = {
        "encw": encw,
        "w0all": w0all,
        "wmisc": wmisc,
        "w1": W1.astype(_BF16),
        "w2": W2.astype(_BF16),
        "w3s": w3s,
        "bias12": bias12,
        "b3h": b3h,
    }

    # ---- per-core prep ----
    props32 = propers_np.astype(np.int32)
    in_maps = []
    for cidx in range(N_CORES):
        shard = np.zeros((PPCT, 4), dtype=np.int32)
        shard[:PPC] = props32[cidx * PPC:(cidx + 1) * PPC]
        order = _order_props(shard, PPC, seed=cidx)
        po = shard[order]                       # [PPCT, 4] in exec order
        is_pad = order >= PPC
        gi = np.concatenate([_wrap_idxs(po[:, k]) for k in range(4)], axis=1)
        tgt0 = np.where(is_pad, DUMP, po[:, 0]).astype(np.int32)
        tgt3 = np.where(is_pad, DUMP, po[:, 3]).astype(np.int32)
        si = np.concatenate([_wrap_idxs(tgt0), _wrap_idxs(tgt3)], axis=1)
        # per-proper coords in exec order: slot i -> (part i%128, b i//128)
        c4 = cflat[po.reshape(-1)].reshape(PPCT, 48)     # [c0|c1|c2|c3]
        cprop = np.ascontiguousarray(
            c4.reshape(98, 128, 48).transpose(1, 0, 2))  # [128, 98, 48]
        in_maps.append({**shared, "gidx": gi, "sidx": si, "cprop": cprop})
    return in_maps


def kernel(coords, propers, encoded, t, answer, W0, b0, W1, b1, W2, b2, W3, b3,
           _trace=False):
    from concourse.bass_utils import run_bass_kernel_spmd

    answer = np.asarray(answer, dtype=np.float32)
    in_maps = _prep_in_maps(coords, propers, encoded, t, answer, W0, b0, W1,
                            b1, W2, b2, W3, b3)
    nc = _get_compiled()
    res = run_bass_kernel_spmd(nc, in_maps, core_ids=list(range(N_CORES)),
                               trace=_trace)
    if _trace:
        kernel.last_exec_ns = res.exec_time_ns
        kernel.last_results = res

    acc = np.zeros((N_ATOMS, 12), dtype=np.float32)
    for cidx in range(N_CORES):
        for q in range(NACC):
            acc += res.results[cidx][f"AA{q}"][:N_ATOMS, :12]
    out = answer + acc.reshape(N_ATOMS, 3, T_STEPS).transpose(0, 2, 1)
    return out.astype(np.float32)


kernel.last_exec_ns = None
kernel.last_results = None


# revision 12
# speedup vs baseline: 1.3747x; 1.3747x over previous
"""Trainium2 Bass kernel for DiffusionPropers (gnn_message_passing).

Strategy: shard the 100K propers across 8 NeuronCores (12544 each incl.
pads).  Per core:
  - The raw `encoded` table (25088 atoms x 128 bf16) lives in SBUF.
  - Per 896-proper chunk: 4 SBUF-source TRANSPOSED dma_gathers fetch the
    endpoint features straight into feature-major layout [128d, 896p]
    (no on-chip transpose needed); the gathers round-robin over the 4
    SWDGE queues so their Q7 descriptor-gen runs concurrently.
  - W0 is folded on-chip: ZT = sum_k W0_k^T @ Ek^T (PE), the geometry
    part enters via a 16-row wmisc matmul, the 128->128->128->2 MLP runs
    feature-major at 448-col granularity in bf16 with Prelu (alpha=1e-3)
    fused into the PSUM evacuations.
  - Per-proper coords are streamed densely from the host (layout prep
    only); dihedral geometry (sin/cos via rsqrt identity, no arctan) is
    computed on DVE in proper-major layout.
  - Corrections dma_scatter_add into 4 rotating DRAM accumulators
    (avoids the WAW serialization of consecutive scatters into one
    tensor).  Race-freedom: the host orders propers so each 896-op
    chunk has all-distinct targets per endpoint; concurrent scatters
    always hit different accumulator tensors.
Host: sums the accumulators of all cores into `answer` (the all-reduce).
"""
import numpy as np
import ml_dtypes

# ---------------- compile-time constants (hardcoded problem shape) --------
N_ATOMS = 25000
NA = 25088              # padded atoms (196 * 128)
P_TOT = 100000
T_STEPS = 4
D = 128
N_CORES = 8
PPC = 12500             # real props per core
PPCT = 12544            # padded props per core (98 tiles of 128)
CH = 896                # props per gather/scatter call (SWDGE ring limit)
NCHUNK = PPCT // CH     # 14
CBLK = CH // 128        # 7
DUMP = NA               # scatter dump row
A_ROWS = NA + 8         # accumulator rows (incl. dump)
A_COLS = 64             # 256B stride for scatter
NACC = 6                # rotating scatter accumulators
LEAKY = 0.001

_BF16 = ml_dtypes.bfloat16

_compiled = None        # cached nc


# ------------------------- host-side helpers ------------------------------

def _wrap_idxs(idx: np.ndarray) -> np.ndarray:
    """[n] int -> [128, n/16] int16, wrapped in 16 partitions, replicated x8."""
    n = idx.shape[0]
    assert n % 16 == 0
    w = idx.reshape(-1, 16).T.astype(np.int16)
    return np.tile(w, (8, 1))


def _order_props(props: np.ndarray, n_real: int, seed: int = 0) -> np.ndarray:
    """Order PPCT props (rows of `props`, first n_real real) so that within
    every aligned CH-chunk the p0 targets are distinct and the p3 targets are
    distinct.  Pads (rows >= n_real) are unconstrained fillers (their scatter
    indices point at the dump row).  Returns a permutation of length PPCT."""
    n = props.shape[0]
    rng = np.random.default_rng(seed)
    for attempt in range(50):
        perm = rng.permutation(n_real)
        buckets: list[list[int]] = [[] for _ in range(NCHUNK)]
        used0: list[set] = [set() for _ in range(NCHUNK)]
        used3: list[set] = [set() for _ in range(NCHUNK)]
        fail = []
        start = 0
        for j in perm:
            a0 = int(props[j, 0])
            a3 = int(props[j, 3])
            for d in range(NCHUNK):
                b = (start + d) % NCHUNK
                if (len(buckets[b]) < CH and a0 not in used0[b]
                        and a3 not in used3[b]):
                    buckets[b].append(int(j))
                    used0[b].add(a0)
                    used3[b].add(a3)
                    break
            else:
                fail.append(int(j))
            start = (start + 1) % NCHUNK
        if fail:
            continue
        pads = list(range(n_real, n))
        for b in range(NCHUNK):
            while len(buckets[b]) < CH:
                buckets[b].append(pads.pop())
        assert not pads
        order = [j for b in buckets for j in b]
        return np.array(order, dtype=np.int64)
    raise RuntimeError("prop ordering failed")


# ------------------------- device kernel build ----------------------------

def _build():
    import concourse.bass as bass
    import concourse.bacc as bacc
    import concourse.mybir as mybir
    import concourse.tile as tile
    from concourse.masks import make_identity
    from concourse.library_config import mlp as mlp_lib

    F32 = mybir.dt.float32
    BF16 = mybir.dt.bfloat16
    I16 = mybir.dt.int16
    AF = mybir.ActivationFunctionType

    nc = bacc.Bacc("TRN2", target_bir_lowering=False, debug=False,
                   num_devices=N_CORES, num_swdge_queues=4)

    # ---- I/O ----
    encw = nc.dram_tensor("encw", [128, 196 * 128], BF16, kind="ExternalInput")
    cprop = nc.dram_tensor("cprop", [128, 98, 48], F32, kind="ExternalInput")
    w0all = nc.dram_tensor("w0all", [D, 512], BF16, kind="ExternalInput")
    wmisc = nc.dram_tensor("wmisc", [16, 512], BF16, kind="ExternalInput")
    w1 = nc.dram_tensor("w1", [D, D], BF16, kind="ExternalInput")
    w2 = nc.dram_tensor("w2", [D, D], BF16, kind="ExternalInput")
    w3s = nc.dram_tensor("w3s", [D, 2], BF16, kind="ExternalInput")
    bias12 = nc.dram_tensor("bias12", [D, 2], F32, kind="ExternalInput")
    b3h = nc.dram_tensor("b3h", [D, 2], F32, kind="ExternalInput")
    gidx = nc.dram_tensor("gidx", [128, 4 * (PPCT // 16)], I16, kind="ExternalInput")
    sidx = nc.dram_tensor("sidx", [128, 2 * (PPCT // 16)], I16, kind="ExternalInput")
    AA = [nc.dram_tensor(f"AA{q}", [A_ROWS, A_COLS], F32, kind="ExternalOutput")
          for q in range(NACC)]

    GI = PPCT // 16     # 784: idx columns per endpoint
    CGI = CH // 16      # 56: idx columns per chunk

    with tile.TileContext(nc) as tc:
        with (
            tc.tile_pool(name="const", bufs=1) as cpool,
        ):
            nc.gpsimd.load_library(mlp_lib)

            # ---- constants ----
            ibf = cpool.tile([128, 128], BF16)
            make_identity(nc, ibf[:])
            if32 = cpool.tile([128, 128], F32)
            make_identity(nc, if32[:])
            id8 = cpool.tile([8, 8], F32)
            make_identity(nc, id8[:])
            zero_b = cpool.tile([128, 1], F32)
            nc.vector.memset(zero_b[:], 0.0)
            eps_b = cpool.tile([128, 1], F32)
            nc.vector.memset(eps_b[:], 1e-12)

            w0t = cpool.tile([D, 512], BF16)
            nc.sync.dma_start(out=w0t[:], in_=w0all[:])
            wmt = cpool.tile([16, 512], BF16)
            nc.sync.dma_start(out=wmt[:], in_=wmisc[:])
            w1t = cpool.tile([D, D], BF16)
            nc.sync.dma_start(out=w1t[:], in_=w1[:])
            w2t = cpool.tile([D, D], BF16)
            nc.sync.dma_start(out=w2t[:], in_=w2[:])
            w3t = cpool.tile([D, 2], BF16)
            nc.sync.dma_start(out=w3t[:], in_=w3s[:])
            b12t = cpool.tile([D, 2], F32)
            nc.sync.dma_start(out=b12t[:], in_=bias12[:])
            b3t = cpool.tile([D, 2], F32)
            nc.sync.dma_start(out=b3t[:], in_=b3h[:])
            gixt = cpool.tile([128, 4 * GI], I16)
            nc.sync.dma_start(out=gixt[:], in_=gidx[:])
            sixt = cpool.tile([128, 2 * GI], I16)
            nc.sync.dma_start(out=sixt[:], in_=sidx[:])

            # encoded table resident in SBUF: atom a -> partition a%128,
            # rank a//128, 256B per rank
            TB = cpool.tile([128, 196 * 128], BF16)
            nc.scalar.dma_start(out=TB[:], in_=encw[:])
            # all per-proper coords [c0|c1|c2|c3] f32, exec order
            cpall = cpool.tile([128, 98, 48], F32)
            nc.sync.dma_start(out=cpall[:], in_=cprop[:])

            # ================= main loop: 14 chunks, software-pipelined ====
            with (
                tc.tile_pool(name="mn", bufs=2) as mpool,
                tc.tile_pool(name="geo", bufs=2) as gpool,
                tc.tile_pool(name="cto", bufs=8) as ctpool,
                tc.tile_pool(name="psb", bufs=1, space="PSUM") as psb,   # big: [128,1024]
                tc.tile_pool(name="pss", bufs=1, space="PSUM") as pss,   # small
            ):
                Gof = {}
                ctof = {}

                def do_gather(c):
                    G = []
                    for k in range(4):
                        g = mpool.tile([128, 1, CH], BF16, tag=f"g{k}", bufs=4)
                        nc.gpsimd.dma_gather(
                            g[:], TB[:],
                            gixt[:, k * GI + c * CGI:k * GI + (c + 1) * CGI],
                            CH, CH, 128, transpose=True,
                            sbuf_tokens_per_rank=128,
                            sbuf_free_dim_per_rank=256,
                            queue_num=1 + (4 * c + k) % 3)
                        G.append(g)
                    Gof[c] = G

                def do_compute(c):
                    G = Gof.pop(c)
                    cco = [cpall[:, c * CBLK:(c + 1) * CBLK, 12 * k:12 * (k + 1)]
                           for k in range(4)]
                    u1 = gpool.tile([128, CBLK, 12], F32, tag="u1")
                    u2 = gpool.tile([128, CBLK, 12], F32, tag="u2")
                    u3 = gpool.tile([128, CBLK, 12], F32, tag="u3")
                    dr = gpool.tile([128, CBLK, 12], F32, tag="dr")
                    nc.vector.tensor_sub(u1[:], cco[1], cco[0])
                    nc.vector.tensor_sub(u2[:], cco[2], cco[1])
                    nc.vector.tensor_sub(u3[:], cco[3], cco[2])
                    nc.vector.tensor_sub(dr[:], cco[0], cco[3])

                    def cross(out, a, b):
                        tmp = gpool.tile([128, CBLK, 4], F32, tag="ctmp")
                        for x in range(3):
                            y, z = (x + 1) % 3, (x + 2) % 3
                            sx, sy, sz = (slice(4 * v, 4 * v + 4) for v in (x, y, z))
                            nc.vector.tensor_mul(tmp[:], a[:, :, sy], b[:, :, sz])
                            nc.vector.tensor_mul(out[:, :, sx], a[:, :, sz], b[:, :, sy])
                            nc.vector.tensor_sub(out[:, :, sx], tmp[:], out[:, :, sx])

                    cr12 = gpool.tile([128, CBLK, 12], F32, tag="cr12")
                    cr23 = gpool.tile([128, CBLK, 12], F32, tag="cr23")
                    cross(cr12, u1, u2)
                    cross(cr23, u2, u3)

                    def dot3(out, a, b, tmp):
                        nc.vector.tensor_mul(tmp[:], a[:], b[:])
                        nc.vector.tensor_add(out[:], tmp[:, :, 0:4], tmp[:, :, 4:8])
                        nc.vector.tensor_add(out[:], out[:], tmp[:, :, 8:12])

                    tmp12 = gpool.tile([128, CBLK, 12], F32, tag="tmp12")
                    n2 = gpool.tile([128, CBLK, 4], F32, tag="n2")
                    dot3(n2, u2, u2, tmp12)
                    nc.scalar.activation(n2[:], n2[:], AF.Sqrt, bias=zero_b[:])
                    sn = gpool.tile([128, CBLK, 4], F32, tag="sn")
                    dot3(sn, u1, cr23, tmp12)
                    nc.vector.tensor_mul(sn[:], sn[:], n2[:])
                    cn = gpool.tile([128, CBLK, 4], F32, tag="cn")
                    dot3(cn, cr12, cr23, tmp12)
                    hy = gpool.tile([128, CBLK, 4], F32, tag="hy")
                    t2 = gpool.tile([128, CBLK, 4], F32, tag="t2")
                    nc.vector.tensor_mul(hy[:], sn[:], sn[:])
                    nc.vector.tensor_mul(t2[:], cn[:], cn[:])
                    nc.vector.tensor_add(hy[:], hy[:], t2[:])
                    nc.scalar.activation(hy[:], hy[:], AF.Sqrt, bias=eps_b[:])
                    rh = gpool.tile([128, CBLK, 4], F32, tag="rh")
                    nc.vector.reciprocal(rh[:], hy[:])
                    dl = gpool.tile([128, CBLK, 4], F32, tag="dl")
                    dot3(dl, dr, dr, tmp12)
                    nc.scalar.activation(dl[:], dl[:], AF.Sqrt, bias=eps_b[:])
                    rdl = gpool.tile([128, CBLK, 4], F32, tag="rdl")
                    nc.vector.reciprocal(rdl[:], dl[:])
                    dh = gpool.tile([128, CBLK, 12], F32, tag="dh")
                    for x in range(3):
                        nc.vector.tensor_mul(dh[:, :, 4 * x:4 * x + 4],
                                             dr[:, :, 4 * x:4 * x + 4], rdl[:])
                    # geo layout: rows f*4+ti after transpose (f: sin,cos,dl,one)
                    geo = gpool.tile([128, CBLK, 16], F32, tag="geo")
                    nc.vector.memset(geo[:], 1.0)
                    nc.vector.tensor_mul(geo[:, :, 0:4], sn[:], rh[:])
                    nc.vector.tensor_mul(geo[:, :, 4:8], cn[:], rh[:])
                    nc.vector.tensor_copy(geo[:, :, 8:12], dl[:])

                    # geoT [16, 896] via per-b PE transposes
                    gtb = gpool.tile([16, CH], BF16, tag="gtb")
                    for b in range(CBLK):
                        smt = pss.tile([128, 128], F32, tag="sm")
                        gtp = smt[0:16, :]
                        nc.tensor.matmul(gtp, lhsT=geo[:, b, :], rhs=if32[:],
                                         is_transpose=True, start=True, stop=True)
                        nc.vector.tensor_copy(gtb[:, b * 128:(b + 1) * 128],
                                              gtp)

                    # ZT = sum_k W0_k^T @ Ek^T  -> [128, 896] bf16 (960 layout)
                    BLKS = [slice(0, 512), slice(512, 896)]
                    zps = psb.tile([128, 1024], F32, tag="hA")
                    for sl in BLKS:
                        for k in range(4):
                            nc.tensor.matmul(zps[:, sl],
                                             lhsT=w0t[:, k * 128:(k + 1) * 128],
                                             rhs=G[k][:, 0, sl],
                                             start=(k == 0), stop=(k == 3))
                    ztb = mpool.tile([128, CH], BF16, tag="ztb")
                    nc.vector.tensor_copy(ztb[:], zps[:, 0:CH])

                    # MLP per ti at 448-col granularity (960-wide tiles with
                    # a dead 448:512 gap so one ACT op covers both banks)
                    x1 = mpool.tile([128, 4, CH], BF16, tag="x1")
                    x2 = mpool.tile([128, 4, CH], BF16, tag="x2")
                    x3 = mpool.tile([128, 4, CH], BF16, tag="x3")
                    # dsb rows: s0_ti at partition 32*ti, s3_ti at 32*ti+1
                    # (quadrant-aligned partition offsets)
                    dsb = gpool.tile([128, CH], F32, tag="dsb")
                    for ti in range(4):
                        h1 = psb.tile([128, 1024], F32, tag="hB")
                        for sl in BLKS:
                            nc.tensor.matmul(h1[:, sl], lhsT=ibf[:],
                                             rhs=ztb[:, sl],
                                             start=True, stop=False)
                            nc.tensor.matmul(h1[:, sl],
                                             lhsT=wmt[:, ti * 128:(ti + 1) * 128],
                                             rhs=gtb[:, sl],
                                             start=False, stop=True)
                        nc.scalar.activation(x1[:, ti, :], h1[:, 0:CH], AF.Prelu,
                                             bias=zero_b[:], alpha=LEAKY)
                        h2 = psb.tile([128, 1024], F32, tag="hC")
                        for sl in BLKS:
                            nc.tensor.matmul(h2[:, sl], lhsT=w1t[:],
                                             rhs=x1[:, ti, sl],
                                             start=True, stop=True)
                        nc.scalar.activation(x2[:, ti, :], h2[:, 0:CH], AF.Prelu,
                                             bias=b12t[:, 0:1], alpha=LEAKY)
                        h3 = psb.tile([128, 1024], F32, tag="hA")
                        for sl in BLKS:
                            nc.tensor.matmul(h3[:, sl], lhsT=w2t[:],
                                             rhs=x2[:, ti, sl],
                                             start=True, stop=True)
                        nc.scalar.activation(x3[:, ti, :], h3[:, 0:CH], AF.Prelu,
                                             bias=b12t[:, 1:2], alpha=LEAKY)
                        for bi in range(2):
                            sl = slice(bi * 448, bi * 448 + 448)
                            dd = pss.tile([2, 448], F32, tag="dd")
                            nc.tensor.matmul(dd[:], lhsT=w3t[:],
                                             rhs=x3[:, ti, sl],
                                             start=True, stop=True)
                            if bi == 0:
                                nc.vector.tensor_copy(
                                    dsb[32 * ti:32 * ti + 2, sl], dd[:])
                            else:
                                nc.scalar.activation(
                                    dsb[32 * ti:32 * ti + 2, sl], dd[:],
                                    AF.Copy)

                    # back to proper-major: dtc [128, 7, 8] (cols e*4+ti)
                    dtc = gpool.tile([128, CBLK, 8], F32, tag="dtc")
                    for b in range(CBLK):
                        dtpf = pss.tile([128, 128], F32, tag="sm")
                        nc.tensor.matmul(dtpf[:],
                                         lhsT=dsb[:, b * 128:(b + 1) * 128],
                                         rhs=if32[:], is_transpose=True,
                                         start=True, stop=True)
                        nc.vector.tensor_copy(dtc[:, b, 0:4], dtpf[:, 0::32])
                        nc.vector.tensor_copy(dtc[:, b, 4:8], dtpf[:, 1::32])

                    c0t = ctpool.tile([128, CBLK, 12], F32, tag="c0t")
                    c3t = ctpool.tile([128, CBLK, 12], F32, tag="c3t")
                    s0 = gpool.tile([128, CBLK, 4], F32, tag="s0")
                    s3 = gpool.tile([128, CBLK, 4], F32, tag="s3")
                    nc.vector.tensor_scalar_add(s0[:], dtc[:, :, 0:4],
                                                b3t[:, 0:1])
                    nc.vector.tensor_scalar_add(s3[:], dtc[:, :, 4:8],
                                                b3t[:, 1:2])
                    for x in range(3):
                        xs = slice(4 * x, 4 * x + 4)
                        nc.vector.tensor_mul(c0t[:, :, xs], dh[:, :, xs], s0[:])
                        nc.vector.tensor_mul(c3t[:, :, xs], dh[:, :, xs], s3[:])
                    ctof[c] = (c0t, c3t)

                def do_scatter(c):
                    # each accumulator is bound to one SWDGE queue so its
                    # scatters are FIFO-serialized (no concurrent CCE adds)
                    c0t, c3t = ctof.pop(c)
                    a0 = (2 * c) % NACC
                    a3 = (2 * c + 1) % NACC
                    nc.gpsimd.dma_scatter_add(
                        AA[a0][:, :12], c0t[:],
                        sixt[:, c * CGI:(c + 1) * CGI],
                        CH, CH, 12, elem_step=A_COLS,
                        queue_num=1 + a0 % 3)
                    nc.gpsimd.dma_scatter_add(
                        AA[a3][:, :12], c3t[:],
                        sixt[:, GI + c * CGI:GI + (c + 1) * CGI],
                        CH, CH, 12, elem_step=A_COLS,
                        queue_num=1 + a3 % 3)

                for c in range(NCHUNK):
                    do_gather(c)
                    if c >= 1:
                        do_compute(c - 1)
                    if c >= 2:
                        do_scatter(c - 2)
                do_compute(NCHUNK - 1)
                do_scatter(NCHUNK - 2)
                do_scatter(NCHUNK - 1)

    nc.compile()
    return nc


def _get_compiled():
    global _compiled
    if _compiled is None:
        _compiled = _build()
    return _compiled


# ------------------------------ entry point -------------------------------

def _prep_in_maps(coords, propers, encoded, t, answer, W0, b0, W1, b1, W2, b2,
                  W3, b3):
    coords = np.asarray(coords, dtype=np.float32)
    propers_np = np.asarray(propers)
    encoded = np.asarray(encoded, dtype=np.float32)
    t = np.asarray(t, dtype=np.float32)
    W0 = np.asarray(W0, dtype=np.float32)
    b0 = np.asarray(b0, dtype=np.float32)
    W1 = np.asarray(W1, dtype=np.float32)
    b1 = np.asarray(b1, dtype=np.float32)
    W2 = np.asarray(W2, dtype=np.float32)
    b2 = np.asarray(b2, dtype=np.float32)
    W3 = np.asarray(W3, dtype=np.float32)
    b3 = np.asarray(b3, dtype=np.float32)

    # ---- shared (replicated) tensors ----
    encp = np.zeros((NA, D), dtype=_BF16)
    encp[:N_ATOMS] = encoded.astype(_BF16)
    # atom a -> partition a%128, rank a//128
    encw = np.ascontiguousarray(
        encp.reshape(196, 128, 128).transpose(1, 0, 2)).reshape(128, 196 * 128)

    # comp-major per-atom coords: cols = comp*4 + ti
    cflat = np.zeros((NA, 12), dtype=np.float32)
    cflat[:N_ATOMS] = coords.transpose(0, 2, 1).reshape(N_ATOMS, 12)

    w0all = np.concatenate([W0[128 * k:128 * (k + 1)] for k in range(4)],
                           axis=1).astype(_BF16)           # [128, 512]
    # geoT rows are f*4+ti (f: sin, cos, dl, one)
    wmisc = np.zeros((16, 512), dtype=np.float32)
    for ti in range(T_STEPS):
        wmisc[0 + ti, ti * 128:(ti + 1) * 128] = W0[513]
        wmisc[4 + ti, ti * 128:(ti + 1) * 128] = W0[514]
        wmisc[8 + ti, ti * 128:(ti + 1) * 128] = W0[515]
        wmisc[12 + ti, ti * 128:(ti + 1) * 128] = b0 + t[ti] * W0[512]
    wmisc = wmisc.astype(_BF16)
    w3s = np.stack([-0.5 * W3[:, 0], 0.5 * W3[:, 1]], axis=1).astype(_BF16)
    bias12 = np.stack([b1, b2], axis=1).astype(np.float32)  # [128, 2]
    b3h = np.zeros((D, 2), dtype=np.float32)
    b3h[:, 0] = -0.5 * b3[0]
    b3h[:, 1] = 0.5 * b3[1]

    shared = {
        "encw": encw,
        "w0all": w0all,
        "wmisc": wmisc,
        "w1": W1.astype(_BF16),
        "w2": W2.astype(_BF16),
        "w3s": w3s,
        "bias12": bias12,
        "b3h": b3h,
    }

    # ---- per-core prep ----
    props32 = propers_np.astype(np.int32)
    in_maps = []
    for cidx in range(N_CORES):
        shard = np.zeros((PPCT, 4), dtype=np.int32)
        shard[:PPC] = props32[cidx * PPC:(cidx + 1) * PPC]
        order = _order_props(shard, PPC, seed=cidx)
        po = shard[order]                       # [PPCT, 4] in exec order
        is_pad = order >= PPC
        gi = np.concatenate([_wrap_idxs(po[:, k]) for k in range(4)], axis=1)
        tgt0 = np.where(is_pad, DUMP, po[:, 0]).astype(np.int32)
        tgt3 = np.where(is_pad, DUMP, po[:, 3]).astype(np.int32)
        si = np.concatenate([_wrap_idxs(tgt0), _wrap_idxs(tgt3)], axis=1)
        # per-proper coords in exec order: slot i -> (part i%128, b i//128)
        c4 = cflat[po.reshape(-1)].reshape(PPCT, 48)     # [c0|c1|c2|c3]
        cprop = np.ascontiguousarray(
            c4.reshape(98, 128, 48).transpose(1, 0, 2))  # [128, 98, 48]
        in_maps.append({**shared, "gidx": gi, "sidx": si, "cprop": cprop})
    return in_maps


def kernel(coords, propers, encoded, t, answer, W0, b0, W1, b1, W2, b2, W3, b3,
           _trace=False):
    from concourse.bass_utils import run_bass_kernel_spmd

    answer = np.asarray(answer, dtype=np.float32)
    in_maps = _prep_in_maps(coords, propers, encoded, t, answer, W0, b0, W1,
                            b1, W2, b2, W3, b3)
    nc = _get_compiled()
    res = run_bass_kernel_spmd(nc, in_maps, core_ids=list(range(N_CORES)),
                               trace=_trace)
    if _trace:
        kernel.last_exec_ns = res.exec_time_ns
        kernel.last_results = res

    acc = np.zeros((N_ATOMS, 12), dtype=np.float32)
    for cidx in range(N_CORES):
        for q in range(NACC):
            acc += res.results[cidx][f"AA{q}"][:N_ATOMS, :12]
    out = answer + acc.reshape(N_ATOMS, 3, T_STEPS).transpose(0, 2, 1)
    return out.astype(np.float32)


kernel.last_exec_ns = None
kernel.last_results = None
